# revision 31
# baseline (speedup 1.0000x reference)
"""Trainium2 Bass kernel for ActiveMatter NPINN PDE loss.

Computes (total, loss_cont, loss_conc, loss_dxx) for the upwind-convection /
diffusion / continuity PINN residuals over inputs u, v, c, Dxx of shape
(4, 22, 256, 256) fp32.

Sharding: 8 cores, core i <- (batch b = i//2, time-half h = i%2).  Each core
handles 10 interior time frames; c/Dxx need a +-1 frame halo (12 frames).
Each core reduces its residual squares to 3 partial sums; the host adds the
8 partial triples and forms the final scalars.

Layout: each 256x256 frame is packed [128 part, 2 blk, 256 w] (+-2 col halo
-> 260 per blk) with rows h = 128*blk + p.  W-shifts are free-dim offset
reads (odd shifts via a one-column-shifted bf16 copy to keep 4B alignment
for the DVE 2x mode); H-shifts/stencils are PE matmuls with circulant
block matrices (main 128x128 band + cross-block wrap correction).

All math 4*DX-scaled; see _build for the fused op list.
"""

import sys

for _p in ("/opt/trn_rl_repo",):
    if _p not in sys.path:
        sys.path.insert(0, _p)

import numpy as np

import concourse.bass as bass
import concourse.bacc as bacc
import concourse.mybir as mybir
from concourse.tile import TileContext
from concourse.bass_utils import run_bass_kernel_spmd

# ---------------------------------------------------------------- constants
B, T, H, W = 4, 22, 256, 256
N_CORES = 8
T_INT = 10           # interior frames per core
T_HALO = T_INT + 2   # c/Dxx frames per core
DX = 10.0 / 256.0
DT = 0.25
D_T = 0.05
BETA = D_T / DX                     # 1.28
KT = 8.0 * DX                       # df_dt coeff on (f_next - f_prev), 4x scale
F32 = mybir.dt.float32
BF = mybir.dt.bfloat16
AL = mybir.AluOpType
AF = mybir.ActivationFunctionType

_CACHE = {}


# ------------------------------------------------------- stencil matrices
def _circ_lhst(taps: dict) -> np.ndarray:
    """[2, 128, 128] lhsT blocks (main, corr) for the periodic row stencil
    out[h] = sum_s taps[s] * f[(h+s) % 256] on 256 rows packed as two
    128-row blocks."""
    M = np.zeros((256, 256), np.float64)
    for s, a in taps.items():
        for h in range(256):
            M[h, (h + s) % 256] += a
    A = M[:128, :128]
    C = M[:128, 128:256]
    assert np.allclose(M[128:, 128:], A) and np.allclose(M[128:, :128], C)
    return np.stack([A.T, C.T]).astype(np.float32)


def _stencil_mats() -> np.ndarray:
    b = 4 * BETA / KT
    mats = [
        _circ_lhst({1: 2.0 / KT, -1: -2.0 / KT}),                # 0 dvy (x2/KT)
        _circ_lhst({0: 1.0, 1: 1.0}),                            # 1 vcn
        _circ_lhst({0: 3.0, -1: -1.0}),                          # 2 gp
        _circ_lhst({1: 3.0, 2: -1.0}),                           # 3 gm
        _circ_lhst({1: b, 0: -2 * b, -1: b}),                    # 4 lape (/KT)
        _circ_lhst({-1: 1.0}),                                   # 5 sm
    ]
    # [6, 2, 128, 128] -> [128, 6, 2, 128]
    return np.ascontiguousarray(np.transpose(np.stack(mats), (2, 0, 1, 3)))


# ------------------------------------------------------------ graph build
def _build():
    nc = bacc.Bacc("TRN2")
    u_ext = nc.declare_dram_parameter("u", [T_INT, H, W], F32, isOutput=False)
    v_ext = nc.declare_dram_parameter("v", [T_INT, H, W], F32, isOutput=False)
    c_ext = nc.declare_dram_parameter("c", [T_HALO, H, W], F32, isOutput=False)
    d_ext = nc.declare_dram_parameter("d", [T_HALO, H, W], F32, isOutput=False)
    sm_ext = nc.declare_dram_parameter("stmat", [128, 6, 2, 128], F32, isOutput=False)
    out_ext = nc.declare_dram_parameter("out", [1, 4], F32, isOutput=True)

    G = 2          # frames per batched group
    NG = T_INT // G

    with TileContext(nc) as tc:
        with (
            tc.tile_pool(name="const", bufs=1) as constp,
            tc.tile_pool(name="frames", bufs=1) as framesp,
            tc.tile_pool(name="scr", bufs=14) as scr,
            tc.tile_pool(name="scr1", bufs=8) as scr1,
            tc.tile_pool(name="psum", bufs=1, space="PSUM") as psp,
        ):
            stmat = constp.tile([128, 6, 2, 128], BF, name="stmat_sb")
            nc.gpsimd.dma_start(out=stmat, in_=sm_ext[:, :, :, :])

            ones = constp.tile([128, 1], F32, name="ones_sb")
            nc.vector.memset(ones, 1.0)

            accs = constp.tile([128, 3, NG], F32, name="accs")

            def dram_frames(ext):
                return ext.rearrange("t (j p) w -> p t j w", p=128)

            du, dv = dram_frames(u_ext), dram_frames(v_ext)
            dc, dd_ = dram_frames(c_ext), dram_frames(d_ext)

            # mega frame tiles: [128, T, 2, 260] (ext) / [128, T, 2, 256] (v)
            uM = framesp.tile([128, T_INT, 2, 260], BF, name="uM")
            vM = framesp.tile([128, T_INT, 2, 256], BF, name="vM")
            cM = framesp.tile([128, T_HALO, 2, 260], BF, name="cM")
            dM = framesp.tile([128, T_HALO, 2, 260], BF, name="dM")
            # per-frame-written, batch-read megas
            vcnM = framesp.tile([128, T_INT, 2, 256], BF, name="vcnM")
            dvsM = framesp.tile([128, T_INT, 2, 256], BF, name="dvsM")
            t2M = framesp.tile([128, 2, T_INT, 2, 256], BF, name="t2M")

            def load_ext(mega, src, t0, n):
                tile = mega[:, t0 : t0 + n]
                nc.gpsimd.dma_start(out=tile[:, :, :, 2:258],
                                    in_=src[:, t0 : t0 + n])
                nc.gpsimd.tensor_copy(tile[:, :, :, 0:2],
                                      tile[:, :, :, 256:258])
                nc.gpsimd.tensor_copy(tile[:, :, :, 258:260],
                                      tile[:, :, :, 2:4])

            for t0 in range(0, T_HALO, 4):
                n = min(4, T_HALO - t0)
                load_ext(cM, dc, t0, n)
                load_ext(dM, dd_, t0, n)
                nu = min(4, max(0, T_INT - t0))
                if nu:
                    load_ext(uM, du, t0, nu)
                    nc.gpsimd.dma_start(out=vM[:, t0 : t0 + nu],
                                        in_=dv[:, t0 : t0 + nu])

            def w(name, frames=G):
                # group-batched scratch (ext-shaped per frame)
                return scr.tile([128, frames, 2, 260], BF, name=name, tag="w")

            def w1(name):
                return scr1.tile([128, 2, 260], BF, name=name, tag="w1")

            def swap_blocks(ap3):
                """View of a [.., 2, N] AP with the two blocks swapped."""
                a = [list(x) for x in ap3.ap]
                a2 = a[:-2] + [[-a[-2][0], 2], a[-1]]
                return bass.AP(tensor=ap3.tensor,
                               offset=ap3.offset + ap3.ap[-2][0], ap=a2)

            def stencil(ps, k, rhs3d, start, stop):
                nc.tensor.matmul(ps[:, :, :], stmat[:, k, 0, :], rhs3d,
                                 start=start, stop=False)
                nc.tensor.matmul(ps[:, :, :], stmat[:, k, 1, :],
                                 swap_blocks(rhs3d), start=False, stop=stop)

            STT = nc.vector.scalar_tensor_tensor
            TT = nc.vector.tensor_tensor

            def rdg(mega, t0, k, n=G):      # batched ext read, k even
                return mega[:, t0 : t0 + n, :, 2 + k : 258 + k]

            def rdog(odd, k, n=G):          # batched odd read (scr tile), k odd
                return odd[:, 0:n, :, 3 + k : 259 + k]

            for g in range(NG):
                tg = g * G
                # ---------------- batched shared chain
                uo = w("uo")
                nc.scalar.copy(uo[:, :, :, 1:260], uM[:, tg : tg + G, :, 0:259])
                uc2 = w("uc2")
                uc2v = uc2[:, :, :, 0:256]
                TT(uc2v, rdg(uM, tg, 0), rdog(uo, 1), AL.add)
                ucp = w("ucp")[:, :, :, 0:256]
                nc.vector.tensor_scalar(ucp, uc2v, 0.0, 1.0 / KT,
                                        AL.max, AL.mult)
                ucm = w("ucm")[:, :, :, 0:256]
                nc.vector.tensor_scalar(ucm, uc2v, 0.0, 1.0 / KT,
                                        AL.min, AL.mult)
                dvx = w("dvx")[:, :, :, 0:256]
                TT(dvx, rdog(uo, 1), rdog(uo, -1), AL.subtract)

                # ---------------- per-frame PSUM-coupled ops
                for ti in range(G):
                    t = tg + ti
                    vsl = vM[:, t]
                    dvy = psp.tile([128, 2, 256], F32, name="dvy", tag="dvy")
                    stencil(dvy, 0, vsl, True, True)
                    STT(dvsM[:, t], dvx[:, ti], 2.0 / KT, dvy, AL.mult, AL.add)

                    vcnp = psp.tile([128, 2, 256], F32, name="vcnp", tag="vcn")
                    stencil(vcnp, 1, vsl, True, True)
                    nc.scalar.mul(vcnM[:, t], vcnp, 1.0 / KT)

                    for fi, fM in ((0, cM), (1, dM)):
                        fsl = fM[:, t + 1, :, 2:258]
                        gp = psp.tile([128, 2, 256], F32, name="gp", tag="gp",
                                      bufs=2)
                        stencil(gp, 2, fsl, True, True)
                        gm = psp.tile([128, 2, 256], F32, name="gm", tag="gm",
                                      bufs=2)
                        stencil(gm, 3, fsl, True, True)
                        p2 = w1("p2")[:, :, 0:256]
                        STT(p2, vcnM[:, t], 0.0, gp, AL.max, AL.mult)
                        q2 = w1("q2")[:, :, 0:256]
                        STT(q2, vcnM[:, t], 0.0, gm, AL.min, AL.mult)
                        psi = w1("psi")[:, :, 0:256]
                        TT(psi, p2, q2, AL.add)

                        py = psp.tile([128, 2, 256], F32, name="py", tag="py",
                                      bufs=2)
                        stencil(py, 5, psi, True, False)
                        stencil(py, 4, fsl, False, True)
                        TT(t2M[:, fi, t], psi, py, AL.subtract)

                # ---------------- batched per-field residual
                for fi, fM in ((0, cM), (1, dM)):
                    fsrc = fM[:, tg + 1 : tg + 1 + G, :, 0:259]
                    fo = w("fo")
                    nc.scalar.copy(fo[:, :, :, 1:260], fsrc)
                    f3o = w("f3o")
                    nc.scalar.mul(f3o[:, :, :, 1:260], fsrc, 3.0)
                    f0b = rdg(fM, tg + 1, 0)
                    f3 = w("f3")[:, :, :, 0:256]
                    nc.vector.tensor_scalar_mul(f3, f0b, 3.0)
                    fp = w("fp")[:, :, :, 0:256]
                    TT(fp, f3, rdog(fo, -1), AL.subtract)
                    fm = w("fm")[:, :, :, 0:256]
                    TT(fm, rdog(f3o, 1), rdg(fM, tg + 1, 2), AL.subtract)
                    p = w("p")[:, :, :, 0:256]
                    TT(p, ucp, fp, AL.mult)
                    q = w("q")[:, :, :, 0:256]
                    TT(q, ucm, fm, AL.mult)
                    s1 = w("s1")[:, :, :, 0:256]
                    TT(s1, p, q, AL.add)
                    dd = w("dd")[:, :, :, 0:256]
                    TT(dd, rdog(fo, 1), f0b, AL.subtract)
                    # phi = phi'/KT = (p+q) - (4*beta/KT)*(f_ip1 - f)
                    phi = w("phi")
                    STT(phi[:, :, :, 2:258], dd, -4.0 * BETA / KT, s1,
                        AL.mult, AL.add)
                    po = w("po")
                    nc.scalar.copy(po[:, :, :, 3:259], phi[:, :, :, 2:258])
                    nc.scalar.copy(po[:, :, :, 2:3], phi[:, :, :, 257:258])
                    cdx = w("cdx")[:, :, :, 0:256]
                    TT(cdx, phi[:, :, :, 2:258], rdog(po, -1), AL.subtract)

                    t3 = w("t3")[:, :, :, 0:256]
                    TT(t3, cdx, t2M[:, fi, tg : tg + G], AL.add)
                    pd = w("pd")[:, :, :, 0:256]
                    TT(pd, f0b, dvsM[:, tg : tg + G], AL.mult)
                    ra = w("ra")[:, :, :, 0:256]
                    TT(ra, t3, pd, AL.subtract)
                    dtt = w("dtt")[:, :, :, 0:256]
                    TT(dtt, rdg(fM, tg + 2, 0), rdg(fM, tg, 0), AL.subtract)
                    rr = w("rr")[:, :, :, 0:256]
                    TT(rr, ra, dtt, AL.add)
                    sqf = w("sqf")[:, :, :, 0:256]
                    nc.scalar.activation(sqf, rr, AF.Square,
                                         accum_out=accs[:, 1 + fi, g : g + 1])

                # batched continuity loss for the group
                sq0 = w("sq0")[:, :, :, 0:256]
                nc.scalar.activation(sq0, dvsM[:, tg : tg + G], AF.Square,
                                     accum_out=accs[:, 0, g : g + 1])

            # ---------------- final reduction to [1, 3]
            red3 = constp.tile([128, 3], F32, name="red3")
            for k in range(3):
                nc.vector.tensor_reduce(red3[:, k : k + 1], accs[:, k, :],
                                        mybir.AxisListType.X, AL.add)
            psr = psp.tile([1, 4], F32, name="psr", tag="dvy")
            nc.tensor.matmul(psr[:, 0:3], ones, red3, start=True, stop=True)
            outt = constp.tile([1, 4], F32, name="outt")
            nc.vector.memset(outt, 0.0)
            nc.scalar.copy(outt[:, 0:3], psr[:, 0:3])
            nc.sync.dma_start(out=out_ext[:, :], in_=outt)

    nc.compile()
    return nc


def _get_nc():
    if "nc" not in _CACHE:
        _CACHE["nc"] = _build()
        _CACHE["stmat"] = _stencil_mats()
    return _CACHE["nc"]


def _make_in_maps(u, v, c, Dxx):
    u = np.ascontiguousarray(np.asarray(u, dtype=np.float32))
    v = np.ascontiguousarray(np.asarray(v, dtype=np.float32))
    c = np.ascontiguousarray(np.asarray(c, dtype=np.float32))
    Dxx = np.ascontiguousarray(np.asarray(Dxx, dtype=np.float32))
    stmat = _CACHE["stmat"]
    in_maps = []
    for i in range(N_CORES):
        b, h = i // 2, i % 2
        t0 = 1 + T_INT * h
        in_maps.append({
            "u": np.ascontiguousarray(u[b, t0 : t0 + T_INT]),
            "v": np.ascontiguousarray(v[b, t0 : t0 + T_INT]),
            "c": np.ascontiguousarray(c[b, t0 - 1 : t0 + T_INT + 1]),
            "d": np.ascontiguousarray(Dxx[b, t0 - 1 : t0 + T_INT + 1]),
            "stmat": stmat,
        })
    return in_maps


def _combine(results):
    s = np.zeros(3, dtype=np.float64)
    for r in results:
        s += np.asarray(r["out"], dtype=np.float64)[0, :3]
    n = B * (T - 2) * H * W
    # everything is scaled by 4*DX/KT = 1/2 on device; KT^2/(16 DX^2) = 4
    loss_cont = 4.0 * s[0] / n
    loss_conc = 4.0 * s[1] / n
    loss_dxx = 4.0 * s[2] / n
    total = loss_cont + loss_conc + loss_dxx
    return np.array([total, loss_cont, loss_conc, loss_dxx], dtype=np.float32)


def kernel(u, v, c, Dxx):
    nc = _get_nc()
    in_maps = _make_in_maps(u, v, c, Dxx)
    res = run_bass_kernel_spmd(nc, in_maps, core_ids=list(range(N_CORES)))
    return _combine(res.results)


if __name__ == "__main__":
    rng = np.random.default_rng(0)
    inputs = {
        "u": rng.standard_normal((B, T, H, W), dtype=np.float32),
        "v": rng.standard_normal((B, T, H, W), dtype=np.float32),
        "c": rng.random((B, T, H, W), dtype=np.float32),
        "Dxx": rng.random((B, T, H, W), dtype=np.float32),
    }
    print(kernel(**inputs))


# revision 32
# speedup vs baseline: 1.2666x; 1.2666x over previous
"""Trainium2 Bass kernel for ActiveMatter NPINN PDE loss.

Computes (total, loss_cont, loss_conc, loss_dxx) for the upwind-convection /
diffusion / continuity PINN residuals over inputs u, v, c, Dxx of shape
(4, 22, 256, 256) fp32.

Sharding: 8 cores, core i <- (batch b = i//2, time-half h = i%2).  Each core
handles 10 interior time frames; c/Dxx need a +-1 frame halo (12 frames).
Each core reduces its residual squares to 3 partial sums; the host adds the
8 partial triples and forms the final scalars.

Layout: each 256x256 frame is packed [128 part, 2 blk, 256 w] (+-2 col halo
-> 260 per blk) with rows h = 128*blk + p.  W-shifts are free-dim offset
reads (odd shifts via a one-column-shifted bf16 copy to keep 4B alignment
for the DVE 2x mode); H-shifts/stencils are PE matmuls with circulant
block matrices (main 128x128 band + cross-block wrap correction).

All math 4*DX-scaled; see _build for the fused op list.
"""

import sys

for _p in ("/opt/trn_rl_repo",):
    if _p not in sys.path:
        sys.path.insert(0, _p)

import numpy as np

import concourse.bass as bass
import concourse.bacc as bacc
import concourse.mybir as mybir
from concourse.tile import TileContext
from concourse.bass_utils import run_bass_kernel_spmd

# ---------------------------------------------------------------- constants
B, T, H, W = 4, 22, 256, 256
N_CORES = 8
T_INT = 10           # interior frames per core
T_HALO = T_INT + 2   # c/Dxx frames per core
DX = 10.0 / 256.0
DT = 0.25
D_T = 0.05
BETA = D_T / DX                     # 1.28
KT = 8.0 * DX                       # df_dt coeff on (f_next - f_prev), 4x scale
F32 = mybir.dt.float32
BF = mybir.dt.bfloat16
AL = mybir.AluOpType
AF = mybir.ActivationFunctionType

_CACHE = {}


# ------------------------------------------------------- stencil matrices
def _circ_lhst(taps: dict) -> np.ndarray:
    """[2, 128, 128] lhsT blocks (main, corr) for the periodic row stencil
    out[h] = sum_s taps[s] * f[(h+s) % 256] on 256 rows packed as two
    128-row blocks."""
    M = np.zeros((256, 256), np.float64)
    for s, a in taps.items():
        for h in range(256):
            M[h, (h + s) % 256] += a
    A = M[:128, :128]
    C = M[:128, 128:256]
    assert np.allclose(M[128:, 128:], A) and np.allclose(M[128:, :128], C)
    return np.stack([A.T, C.T]).astype(np.float32)


def _stencil_mats() -> np.ndarray:
    b = 4 * BETA / KT
    mats = [
        _circ_lhst({1: 2.0 / KT, -1: -2.0 / KT}),                # 0 dvy (x2/KT)
        _circ_lhst({0: 1.0, 1: 1.0}),                            # 1 vcn
        _circ_lhst({0: 3.0, -1: -1.0}),                          # 2 gp
        _circ_lhst({1: 3.0, 2: -1.0}),                           # 3 gm
        _circ_lhst({1: b, 0: -2 * b, -1: b}),                    # 4 lape (/KT)
        _circ_lhst({-1: 1.0}),                                   # 5 sm
    ]
    # [6, 2, 128, 128] -> [128, 6, 2, 128]
    return np.ascontiguousarray(np.transpose(np.stack(mats), (2, 0, 1, 3)))


# ------------------------------------------------------------ graph build
def _build():
    nc = bacc.Bacc("TRN2")
    u_ext = nc.declare_dram_parameter("u", [T_INT, H, W], F32, isOutput=False)
    v_ext = nc.declare_dram_parameter("v", [T_INT, H, W], F32, isOutput=False)
    c_ext = nc.declare_dram_parameter("c", [T_HALO, H, W], F32, isOutput=False)
    d_ext = nc.declare_dram_parameter("d", [T_HALO, H, W], F32, isOutput=False)
    sm_ext = nc.declare_dram_parameter("stmat", [128, 6, 2, 128], F32, isOutput=False)
    out_ext = nc.declare_dram_parameter("out", [1, 4], F32, isOutput=True)

    G = 5          # frames per batched group
    NG = T_INT // G

    with TileContext(nc) as tc:
        with (
            tc.tile_pool(name="const", bufs=1) as constp,
            tc.tile_pool(name="frames", bufs=1) as framesp,
            tc.tile_pool(name="scr", bufs=14) as scr,
            tc.tile_pool(name="scr1", bufs=8) as scr1,
            tc.tile_pool(name="psum", bufs=1, space="PSUM") as psp,
        ):
            stmat = constp.tile([128, 6, 2, 128], BF, name="stmat_sb")
            nc.gpsimd.dma_start(out=stmat, in_=sm_ext[:, :, :, :])

            ones = constp.tile([128, 1], F32, name="ones_sb")
            nc.vector.memset(ones, 1.0)

            accs = constp.tile([128, 3, NG], F32, name="accs")

            def dram_frames(ext):
                return ext.rearrange("t (j p) w -> p t j w", p=128)

            du, dv = dram_frames(u_ext), dram_frames(v_ext)
            dc, dd_ = dram_frames(c_ext), dram_frames(d_ext)

            # mega frame tiles: [128, T, 2, 260] (ext) / [128, T, 2, 256] (v)
            uM = framesp.tile([128, T_INT, 2, 260], BF, name="uM")
            vM = framesp.tile([128, T_INT, 2, 256], BF, name="vM")
            cM = framesp.tile([128, T_HALO, 2, 260], BF, name="cM")
            dM = framesp.tile([128, T_HALO, 2, 260], BF, name="dM")
            # per-frame-written, batch-read megas
            vcnM = framesp.tile([128, T_INT, 2, 256], BF, name="vcnM")
            dvsM = framesp.tile([128, T_INT, 2, 256], BF, name="dvsM")
            t2M = framesp.tile([128, 2, T_INT, 2, 256], BF, name="t2M")

            def load_ext(mega, src, t0, n):
                tile = mega[:, t0 : t0 + n]
                nc.gpsimd.dma_start(out=tile[:, :, :, 2:258],
                                    in_=src[:, t0 : t0 + n])
                nc.gpsimd.tensor_copy(tile[:, :, :, 0:2],
                                      tile[:, :, :, 256:258])
                nc.gpsimd.tensor_copy(tile[:, :, :, 258:260],
                                      tile[:, :, :, 2:4])

            for t0 in range(0, T_HALO, 4):
                n = min(4, T_HALO - t0)
                load_ext(cM, dc, t0, n)
                load_ext(dM, dd_, t0, n)
                nu = min(4, max(0, T_INT - t0))
                if nu:
                    load_ext(uM, du, t0, nu)
                    nc.gpsimd.dma_start(out=vM[:, t0 : t0 + nu],
                                        in_=dv[:, t0 : t0 + nu])

            def w(name, frames=G):
                # group-batched scratch (ext-shaped per frame)
                return scr.tile([128, frames, 2, 260], BF, name=name, tag="w")

            def w1(name):
                return scr1.tile([128, 2, 260], BF, name=name, tag="w1")

            def swap_blocks(ap3):
                """View of a [.., 2, N] AP with the two blocks swapped."""
                a = [list(x) for x in ap3.ap]
                a2 = a[:-2] + [[-a[-2][0], 2], a[-1]]
                return bass.AP(tensor=ap3.tensor,
                               offset=ap3.offset + ap3.ap[-2][0], ap=a2)

            def stencil(ps, k, rhs3d, start, stop):
                nc.tensor.matmul(ps[:, :, :], stmat[:, k, 0, :], rhs3d,
                                 start=start, stop=False)
                nc.tensor.matmul(ps[:, :, :], stmat[:, k, 1, :],
                                 swap_blocks(rhs3d), start=False, stop=stop)

            STT = nc.vector.scalar_tensor_tensor
            TT = nc.vector.tensor_tensor

            def rdg(mega, t0, k, n=G):      # batched ext read, k even
                return mega[:, t0 : t0 + n, :, 2 + k : 258 + k]

            def rdog(odd, k, n=G):          # batched odd read (scr tile), k odd
                return odd[:, 0:n, :, 3 + k : 259 + k]

            for g in range(NG):
                tg = g * G
                # ---------------- batched shared chain
                uo = w("uo")
                nc.scalar.copy(uo[:, :, :, 1:260], uM[:, tg : tg + G, :, 0:259])
                uc2 = w("uc2")
                uc2v = uc2[:, :, :, 0:256]
                TT(uc2v, rdg(uM, tg, 0), rdog(uo, 1), AL.add)
                ucp = w("ucp")[:, :, :, 0:256]
                nc.vector.tensor_scalar(ucp, uc2v, 0.0, 1.0 / KT,
                                        AL.max, AL.mult)
                ucm = w("ucm")[:, :, :, 0:256]
                nc.vector.tensor_scalar(ucm, uc2v, 0.0, 1.0 / KT,
                                        AL.min, AL.mult)
                dvx = w("dvx")[:, :, :, 0:256]
                TT(dvx, rdog(uo, 1), rdog(uo, -1), AL.subtract)

                # ---------------- per-frame PSUM-coupled ops
                for ti in range(G):
                    t = tg + ti
                    vsl = vM[:, t]
                    dvy = psp.tile([128, 2, 256], F32, name="dvy", tag="dvy")
                    stencil(dvy, 0, vsl, True, True)
                    STT(dvsM[:, t], dvx[:, ti], 2.0 / KT, dvy, AL.mult, AL.add)

                    vcnp = psp.tile([128, 2, 256], F32, name="vcnp", tag="vcn")
                    stencil(vcnp, 1, vsl, True, True)
                    nc.scalar.mul(vcnM[:, t], vcnp, 1.0 / KT)

                    for fi, fM in ((0, cM), (1, dM)):
                        fsl = fM[:, t + 1, :, 2:258]
                        gp = psp.tile([128, 2, 256], F32, name="gp", tag="gp",
                                      bufs=2)
                        stencil(gp, 2, fsl, True, True)
                        gm = psp.tile([128, 2, 256], F32, name="gm", tag="gm",
                                      bufs=2)
                        stencil(gm, 3, fsl, True, True)
                        p2 = w1("p2")[:, :, 0:256]
                        STT(p2, vcnM[:, t], 0.0, gp, AL.max, AL.mult)
                        q2 = w1("q2")[:, :, 0:256]
                        STT(q2, vcnM[:, t], 0.0, gm, AL.min, AL.mult)
                        psi = w1("psi")[:, :, 0:256]
                        TT(psi, p2, q2, AL.add)

                        py = psp.tile([128, 2, 256], F32, name="py", tag="py",
                                      bufs=2)
                        stencil(py, 5, psi, True, False)
                        stencil(py, 4, fsl, False, True)
                        TT(t2M[:, fi, t], psi, py, AL.subtract)

                # ---------------- batched per-field residual
                for fi, fM in ((0, cM), (1, dM)):
                    fsrc = fM[:, tg + 1 : tg + 1 + G, :, 0:259]
                    fo = w("fo")
                    nc.scalar.copy(fo[:, :, :, 1:260], fsrc)
                    f3o = w("f3o")
                    nc.scalar.mul(f3o[:, :, :, 1:260], fsrc, 3.0)
                    f0b = rdg(fM, tg + 1, 0)
                    f3 = w("f3")[:, :, :, 0:256]
                    nc.vector.tensor_scalar_mul(f3, f0b, 3.0)
                    fp = w("fp")[:, :, :, 0:256]
                    TT(fp, f3, rdog(fo, -1), AL.subtract)
                    fm = w("fm")[:, :, :, 0:256]
                    TT(fm, rdog(f3o, 1), rdg(fM, tg + 1, 2), AL.subtract)
                    p = w("p")[:, :, :, 0:256]
                    TT(p, ucp, fp, AL.mult)
                    q = w("q")[:, :, :, 0:256]
                    TT(q, ucm, fm, AL.mult)
                    s1 = w("s1")[:, :, :, 0:256]
                    TT(s1, p, q, AL.add)
                    dd = w("dd")[:, :, :, 0:256]
                    TT(dd, rdog(fo, 1), f0b, AL.subtract)
                    # phi = phi'/KT = (p+q) - (4*beta/KT)*(f_ip1 - f)
                    phi = w("phi")
                    STT(phi[:, :, :, 2:258], dd, -4.0 * BETA / KT, s1,
                        AL.mult, AL.add)
                    po = w("po")
                    nc.scalar.copy(po[:, :, :, 3:259], phi[:, :, :, 2:258])
                    nc.scalar.copy(po[:, :, :, 2:3], phi[:, :, :, 257:258])
                    cdx = w("cdx")[:, :, :, 0:256]
                    TT(cdx, phi[:, :, :, 2:258], rdog(po, -1), AL.subtract)

                    t3 = w("t3")[:, :, :, 0:256]
                    TT(t3, cdx, t2M[:, fi, tg : tg + G], AL.add)
                    pd = w("pd")[:, :, :, 0:256]
                    TT(pd, f0b, dvsM[:, tg : tg + G], AL.mult)
                    ra = w("ra")[:, :, :, 0:256]
                    TT(ra, t3, pd, AL.subtract)
                    dtt = w("dtt")[:, :, :, 0:256]
                    TT(dtt, rdg(fM, tg + 2, 0), rdg(fM, tg, 0), AL.subtract)
                    rr = w("rr")[:, :, :, 0:256]
                    TT(rr, ra, dtt, AL.add)
                    sqf = w("sqf")[:, :, :, 0:256]
                    nc.scalar.activation(sqf, rr, AF.Square,
                                         accum_out=accs[:, 1 + fi, g : g + 1])

                # batched continuity loss for the group
                sq0 = w("sq0")[:, :, :, 0:256]
                nc.scalar.activation(sq0, dvsM[:, tg : tg + G], AF.Square,
                                     accum_out=accs[:, 0, g : g + 1])

            # ---------------- final reduction to [1, 3]
            red3 = constp.tile([128, 3], F32, name="red3")
            for k in range(3):
                nc.vector.tensor_reduce(red3[:, k : k + 1], accs[:, k, :],
                                        mybir.AxisListType.X, AL.add)
            psr = psp.tile([1, 4], F32, name="psr", tag="dvy")
            nc.tensor.matmul(psr[:, 0:3], ones, red3, start=True, stop=True)
            outt = constp.tile([1, 4], F32, name="outt")
            nc.vector.memset(outt, 0.0)
            nc.scalar.copy(outt[:, 0:3], psr[:, 0:3])
            nc.sync.dma_start(out=out_ext[:, :], in_=outt)

    nc.compile()
    return nc


def _get_nc():
    if "nc" not in _CACHE:
        _CACHE["nc"] = _build()
        _CACHE["stmat"] = _stencil_mats()
    return _CACHE["nc"]


def _make_in_maps(u, v, c, Dxx):
    u = np.ascontiguousarray(np.asarray(u, dtype=np.float32))
    v = np.ascontiguousarray(np.asarray(v, dtype=np.float32))
    c = np.ascontiguousarray(np.asarray(c, dtype=np.float32))
    Dxx = np.ascontiguousarray(np.asarray(Dxx, dtype=np.float32))
    stmat = _CACHE["stmat"]
    in_maps = []
    for i in range(N_CORES):
        b, h = i // 2, i % 2
        t0 = 1 + T_INT * h
        in_maps.append({
            "u": np.ascontiguousarray(u[b, t0 : t0 + T_INT]),
            "v": np.ascontiguousarray(v[b, t0 : t0 + T_INT]),
            "c": np.ascontiguousarray(c[b, t0 - 1 : t0 + T_INT + 1]),
            "d": np.ascontiguousarray(Dxx[b, t0 - 1 : t0 + T_INT + 1]),
            "stmat": stmat,
        })
    return in_maps


def _combine(results):
    s = np.zeros(3, dtype=np.float64)
    for r in results:
        s += np.asarray(r["out"], dtype=np.float64)[0, :3]
    n = B * (T - 2) * H * W
    # everything is scaled by 4*DX/KT = 1/2 on device; KT^2/(16 DX^2) = 4
    loss_cont = 4.0 * s[0] / n
    loss_conc = 4.0 * s[1] / n
    loss_dxx = 4.0 * s[2] / n
    total = loss_cont + loss_conc + loss_dxx
    return np.array([total, loss_cont, loss_conc, loss_dxx], dtype=np.float32)


def kernel(u, v, c, Dxx):
    nc = _get_nc()
    in_maps = _make_in_maps(u, v, c, Dxx)
    res = run_bass_kernel_spmd(nc, in_maps, core_ids=list(range(N_CORES)))
    return _combine(res.results)


if __name__ == "__main__":
    rng = np.random.default_rng(0)
    inputs = {
        "u": rng.standard_normal((B, T, H, W), dtype=np.float32),
        "v": rng.standard_normal((B, T, H, W), dtype=np.float32),
        "c": rng.random((B, T, H, W), dtype=np.float32),
        "Dxx": rng.random((B, T, H, W), dtype=np.float32),
    }
    print(kernel(**inputs))
